# revision 13
# baseline (speedup 1.0000x reference)
"""Trainium2 Bass kernel for nn_CLRBP_23124103922240.

Math: scores[s, c] = x[s] . W[c] + b[c], softmax over 16 classes, where
W[c] = g * tile4x4(A1[c]) + (1-g) * A2[c],
A1[c] = u1 u1^T - v1 v1^T (64x64, rank 8), A2[c] = u2 u2^T - v2 v2^T
(256x256, rank 2), g = sigmoid(l[0]).

Strategy (dense-W, X-stationary):
  - x is cast to fp16 on host (measured output rel-err 4.3e-3, gate 2e-2)
    and re-laid out per core as xt[p, q, s] = x[s, q*128 + p]: the flat
    65536-pixel axis is split into 512 chunks of 128; the DMA stream is
    16.8 MB/core, half of f32.
  - For each chunk q the X block [128 pixels, 128 samples] is the
    *stationary* matmul operand; two [128, 16] moving operands (the g*A1
    table slice and the (1-g)*W2 slice) accumulate scores [128 samples,
    16 classes] directly in PSUM.  No per-sample vector work at all.
  - W1-tiled never materializes: tile4x4 means the moving slice for chunk
    (m, nh) is g*A1[c, m%64, p%64], read from a host-precomputed
    [128, 64, 16] table (p-duplicated so no partition wrap is needed).
  - W2 (dense [65536, 16]) is generated on device from the rank-2 factors:
    for each row m and column half nh, out[p, c] = sum_k st2[k, nh*128+p]
    * dg[k, m, c] with st2 = [u2 | v2] and dg the per-m diag-packed
    (1-g)-scaled factors; 512 tiny matmuls + 16 PSUM->SBUF copies.
  - Bias enters as a K=1 matmul (ones x b); softmax on [128, 16] f32.

Data-parallel over 8 NeuronCores: batch 1024 -> 128 samples per core.
"""

import numpy as np

import concourse.bacc as bacc
import concourse.mybir as mybir
import concourse.tile as tile
from concourse.bass_utils import run_bass_kernel_spmd

N_CORES = 8
B, D, C = 1024, 256, 16
BL = B // N_CORES        # 128 samples per core
NQ = (D * D) // 128      # 512 pixel chunks
G = 16                   # chunks per x DMA group
NG = NQ // G             # 32 groups
MBLK = 32                # W2-gen rows per PSUM bank round

F16 = mybir.dt.float16
F32 = mybir.dt.float32
AOP = mybir.AluOpType
AFT = mybir.ActivationFunctionType
AXL = mybir.AxisListType

_cache = {}


def _build():
    if "nc" in _cache:
        return _cache["nc"]

    nc = bacc.Bacc("TRN2", target_bir_lowering=False, debug=False,
                   num_devices=N_CORES)

    xt_d = nc.dram_tensor("xt", [128, NQ, BL], F16, kind="ExternalInput").ap()
    st1_d = nc.dram_tensor("st1", [128, 128], F16, kind="ExternalInput").ap()
    val1_d = nc.dram_tensor("val1", [128, 64], F16, kind="ExternalInput").ap()
    msk1_d = nc.dram_tensor("msk1", [128, C], F16, kind="ExternalInput").ap()
    st2_d = nc.dram_tensor("st2", [32, 2, 128], F16, kind="ExternalInput").ap()
    val2_d = nc.dram_tensor("val2", [32, D], F16, kind="ExternalInput").ap()
    msk2_d = nc.dram_tensor("msk2", [32, C], F16, kind="ExternalInput").ap()
    ob_d = nc.dram_tensor("ob", [1, BL + C], F16, kind="ExternalInput").ap()
    out_d = nc.dram_tensor("probs", [BL, C], F32, kind="ExternalOutput").ap()

    with tile.TileContext(nc) as tc:
        with (
            tc.tile_pool(name="consts", bufs=1) as consts,
            tc.tile_pool(name="xp", bufs=8) as xpool,
            tc.tile_pool(name="fin", bufs=1) as fin,
            tc.tile_pool(name="gps", bufs=2, space="PSUM") as gpspool,
            tc.tile_pool(name="sc", bufs=1, space="PSUM") as scpool,
        ):
            # group schedule: big groups, tapered tail so the end-of-stream
            # drain only covers a couple of chunks
            sizes = [G] * (NQ // G - 1) + [8, 4, 2, 2]
            starts = [sum(sizes[:i]) for i in range(len(sizes))]

            # x DMA stream first so the big transfer starts at t=0; the
            # W2-gen consts (dg, st2) go right behind group 0 so generation
            # starts early
            pre = {}
            xt = xpool.tile([128, sizes[0], BL], F16, tag="xt")
            nc.sync.dma_start(out=xt, in_=xt_d[:, 0:sizes[0], :])
            pre[0] = xt

            st2 = consts.tile([32, 2, 128], F16)
            nc.sync.dma_start(out=st2, in_=st2_d)
            val2 = consts.tile([32, D], F16)
            nc.sync.dma_start(out=val2, in_=val2_d)
            msk2 = consts.tile([32, C], F16)
            nc.sync.dma_start(out=msk2, in_=msk2_d)

            xt = xpool.tile([128, sizes[1], BL], F16, tag="xt")
            nc.sync.dma_start(out=xt, in_=xt_d[:, starts[1]:starts[1] + sizes[1], :])
            pre[1] = xt

            st1 = consts.tile([128, 128], F16)
            nc.sync.dma_start(out=st1, in_=st1_d)
            val1 = consts.tile([128, 64], F16)
            nc.sync.dma_start(out=val1, in_=val1_d)
            msk1 = consts.tile([128, C], F16)
            nc.sync.dma_start(out=msk1, in_=msk1_d)
            ob = consts.tile([1, BL + C], F16)
            nc.sync.dma_start(out=ob, in_=ob_d)

            # ---- A1 table generation (g * A1[c, j, p%64] at [p, j, c]) ----
            # dg1[k, j, c] = val1[k, j] * msk1[k, c], then
            # a1[p, j, c] = sum_k st1[k, p] * dg1[k, j, c]
            dg1 = consts.tile([128, 64, C], F16)
            nc.vector.scalar_tensor_tensor(
                out=dg1,
                in0=val1.rearrange("p (m c) -> p m c", c=1)
                    .broadcast_to([128, 64, C]),
                scalar=1.0,
                in1=msk1.rearrange("p (m c) -> p m c", m=1)
                    .broadcast_to([128, 64, C]),
                op0=AOP.mult, op1=AOP.mult)
            a1 = consts.tile([128, 64, C], F16)
            for h in range(2):
                a1ps = gpspool.tile([128, 32, C], F32)
                nc.tensor.matmul(a1ps, st1, dg1[:, h * 32:(h + 1) * 32, :],
                                 start=True, stop=True)
                dst = a1[:, h * 32:(h + 1) * 32, :]
                if h == 0:
                    nc.scalar.copy(dst, a1ps)
                else:
                    nc.vector.tensor_scalar_add(dst, a1ps, 0.0)

            # ---- W2 dense generation: w2sb[p, nh, m, c] ----
            # dg[k, m, c] = val2[k, m] * msk2[k, c] (expanded per m-block)
            dg = consts.tile([32, D, C], F16)
            w2sb = consts.tile([128, 2, D, C], F16)
            for mblk in range(D // MBLK):
                ms = slice(mblk * MBLK, (mblk + 1) * MBLK)
                nc.vector.scalar_tensor_tensor(
                    out=dg[:, ms, :],
                    in0=val2[:, ms].rearrange("p (m c) -> p m c", c=1)
                        .broadcast_to([32, MBLK, C]),
                    scalar=1.0,
                    in1=msk2.rearrange("p (m c) -> p m c", m=1)
                        .broadcast_to([32, MBLK, C]),
                    op0=AOP.mult, op1=AOP.mult)
                for nh in range(2):
                    gps = gpspool.tile([128, MBLK, C], F32)
                    nc.tensor.matmul(gps, st2[:, nh, :], dg[:, ms, :],
                                     start=True, stop=True)
                    dst = w2sb[:, nh, ms, :]
                    if nh == 0:
                        nc.scalar.copy(dst, gps)
                    else:
                        nc.vector.tensor_scalar_add(dst, gps, 0.0)

            # ---- main pass: scores accumulate over all 512 chunks ----
            sc = scpool.tile([BL, C], F32)
            first = True
            for gi in range(len(sizes)):
                if gi in pre:
                    xt = pre[gi]
                else:
                    xt = xpool.tile([128, sizes[gi], BL], F16, tag="xt")
                    nc.sync.dma_start(
                        out=xt,
                        in_=xt_d[:, starts[gi]:starts[gi] + sizes[gi], :])
                for t in range(sizes[gi]):
                    q = starts[gi] + t
                    m, nh = q // 2, q % 2
                    nc.tensor.matmul(sc, xt[:, t, :], w2sb[:, nh, m, :],
                                     start=first, stop=False)
                    first = False
                    nc.tensor.matmul(sc, xt[:, t, :], a1[:, m % 64, :],
                                     start=False, stop=False)
            nc.tensor.matmul(sc, ob[:, 0:BL], ob[:, BL:BL + C],
                             start=False, stop=True)

            # ---- softmax over the 16 free elements ----
            negmax = fin.tile([BL, 1], F32)
            nc.vector.tensor_reduce(out=negmax, in_=sc, axis=AXL.X,
                                    op=AOP.max, negate=True)
            e = fin.tile([BL, C], F32)
            sume = fin.tile([BL, 1], F32)
            nc.scalar.activation(out=e, in_=sc, func=AFT.Exp, bias=negmax,
                                 scale=1.0, accum_out=sume)
            rec = fin.tile([BL, 1], F32)
            nc.vector.reciprocal(rec, sume)
            probs = fin.tile([BL, C], F32)
            nc.vector.tensor_scalar_mul(probs, e, rec)
            nc.sync.dma_start(out=out_d, in_=probs)

    nc.compile()
    _cache["nc"] = nc
    return nc


def _host_prep(inputs, w1, w2, l, b):
    inputs = np.asarray(inputs, dtype=np.float32)
    w1 = np.asarray(w1, dtype=np.float32)
    w2 = np.asarray(w2, dtype=np.float32)
    l = np.asarray(l, dtype=np.float32)
    b = np.asarray(b, dtype=np.float32)

    g = np.float32(1.0 / (1.0 + np.exp(-np.float32(l[0]))))

    # A1 gen consts: st1[k=(c,r), p] = w1[c, p%64, r];
    # val1[k, j] = sign_r * g * w1[c, j, r]; msk1[k, c'] = (c' == k//8)
    w1t = w1.transpose(0, 2, 1)                                # [c, r, j]
    w1r = w1t.reshape(128, 64)
    st1 = np.concatenate([w1r, w1r], axis=1).astype(np.float16)
    signs = np.array([-1.0] * 4 + [1.0] * 4, np.float32)
    val1 = (w1t * g * signs[None, :, None]).reshape(128, 64)
    val1 = val1.astype(np.float16)
    msk1 = (np.arange(128)[:, None] // 8 ==
            np.arange(C)[None, :]).astype(np.float16)

    # W2 gen consts: st2[k, n] = (u2|v2)[k, n];
    # val2[k, m] = +-(1-g) * (u2|v2)[k, m]; msk2[k, c'] = (c' == k%16)
    u2, v2 = w2[:, :, 1], w2[:, :, 0]                          # [16, 256]
    st2 = np.concatenate([u2, v2], axis=0).reshape(32, 2, 128)
    st2 = st2.astype(np.float16)
    val2 = np.concatenate([(1.0 - g) * u2, -(1.0 - g) * v2], axis=0)
    val2 = val2.astype(np.float16)
    msk2 = (np.arange(32)[:, None] % 16 ==
            np.arange(C)[None, :]).astype(np.float16)

    ob = np.zeros((1, BL + C), np.float32)
    ob[0, :BL] = 1.0
    ob[0, BL:] = b
    ob = ob.astype(np.float16)

    # x: [1024, 256, 256] -> per-core xt[p, q, s] = x[s, q*128 + p]
    xt_all = inputs.astype(np.float16).reshape(N_CORES, BL, NQ, 128)
    xt_all = np.ascontiguousarray(xt_all.transpose(0, 3, 2, 1))

    shared = dict(st1=st1, val1=val1, msk1=msk1, st2=st2, val2=val2,
                  msk2=msk2, ob=ob)
    in_maps = []
    for core in range(N_CORES):
        m = dict(shared)
        m["xt"] = xt_all[core]
        in_maps.append(m)
    return in_maps


def kernel(inputs, w1, w2, l, b, _trace=False):
    nc = _build()
    in_maps = _host_prep(inputs, w1, w2, l, b)
    res = run_bass_kernel_spmd(nc, in_maps, core_ids=list(range(N_CORES)),
                               trace=_trace)
    out = np.concatenate([r["probs"] for r in res.results], axis=0)
    if _trace:
        kernel.last_results = res
    return out


# revision 19
# speedup vs baseline: 1.0471x; 1.0471x over previous
"""Trainium2 Bass kernel for nn_CLRBP_23124103922240.

Math: scores[s, c] = x[s] . W[c] + b[c], softmax over 16 classes, where
W[c] = g * tile4x4(A1[c]) + (1-g) * A2[c],
A1[c] = u1 u1^T - v1 v1^T (64x64, rank 8), A2[c] = u2 u2^T - v2 v2^T
(256x256, rank 2), g = sigmoid(l[0]).

Strategy (dense-W, X-stationary):
  - x is cast to fp16 on host (measured output rel-err 4.3e-3, gate 2e-2)
    and re-laid out per core as xt[p, q, s] = x[s, q*128 + p]: the flat
    65536-pixel axis is split into 512 chunks of 128; the DMA stream is
    16.8 MB/core, half of f32.
  - For each chunk q the X block [128 pixels, 128 samples] is the
    *stationary* matmul operand; two [128, 16] moving operands (the g*A1
    table slice and the (1-g)*W2 slice) accumulate scores [128 samples,
    16 classes] directly in PSUM.  No per-sample vector work at all.
  - W1-tiled never materializes: tile4x4 means the moving slice for chunk
    (m, nh) is g*A1[c, m%64, p%64], read from a host-precomputed
    [128, 64, 16] table (p-duplicated so no partition wrap is needed).
  - W2 (dense [65536, 16]) is generated on device from the rank-2 factors:
    for each row m and column half nh, out[p, c] = sum_k st2[k, nh*128+p]
    * dg[k, m, c] with st2 = [u2 | v2] and dg the per-m diag-packed
    (1-g)-scaled factors; 512 tiny matmuls + 16 PSUM->SBUF copies.
  - Bias enters as a K=1 matmul (ones x b); softmax on [128, 16] f32.

Data-parallel over 8 NeuronCores: batch 1024 -> 128 samples per core.
"""

import numpy as np

import concourse.bacc as bacc
import concourse.mybir as mybir
import concourse.tile as tile
from concourse.bass_utils import run_bass_kernel_spmd

N_CORES = 8
B, D, C = 1024, 256, 16
BL = B // N_CORES        # 128 samples per core
NQ = (D * D) // 128      # 512 pixel chunks
G = 16                   # chunks per x DMA group
NG = NQ // G             # 32 groups
MBLK = 32                # W2-gen rows per PSUM bank round

F16 = mybir.dt.float16
F32 = mybir.dt.float32
AOP = mybir.AluOpType
AFT = mybir.ActivationFunctionType
AXL = mybir.AxisListType

_cache = {}


def _build():
    if "nc" in _cache:
        return _cache["nc"]

    nc = bacc.Bacc("TRN2", target_bir_lowering=False, debug=False,
                   num_devices=N_CORES)

    xt_d = nc.dram_tensor("xt", [128, NQ, BL], F16, kind="ExternalInput").ap()
    # packed consts: cpa [:, 0:128]=st1, [:,128:192]=val1, [:,192:208]=msk1
    # cpb [0:32, 0:256]=st2, [:,256:512]=val2, [:,512:528]=msk2,
    #     [0:1, 528:672]=ones|bias
    cpa_d = nc.dram_tensor("cpa", [128, 256], F16, kind="ExternalInput").ap()
    cpb_d = nc.dram_tensor("cpb", [32, 672], F16, kind="ExternalInput").ap()
    out_d = nc.dram_tensor("probs", [BL, C], F32, kind="ExternalOutput").ap()

    with tile.TileContext(nc) as tc:
        with (
            tc.tile_pool(name="consts", bufs=1) as consts,
            tc.tile_pool(name="xp", bufs=8) as xpool,
            tc.tile_pool(name="fin", bufs=1) as fin,
            tc.tile_pool(name="gps", bufs=2, space="PSUM") as gpspool,
            tc.tile_pool(name="sc", bufs=1, space="PSUM") as scpool,
        ):
            # group schedule: big groups, tapered tail so the end-of-stream
            # drain only covers a couple of chunks
            sizes = [G] * (NQ // G - 1) + [8, 4, 2, 2]
            starts = [sum(sizes[:i]) for i in range(len(sizes))]

            # x DMA stream first so the big transfer starts at t=0; the
            # W2-gen consts (dg, st2) go right behind group 0 so generation
            # starts early
            pre = {}
            xt = xpool.tile([128, sizes[0], BL], F16, tag="xt")
            nc.sync.dma_start(out=xt, in_=xt_d[:, 0:sizes[0], :])
            pre[0] = xt

            cpa = consts.tile([128, 256], F16)
            nc.sync.dma_start(out=cpa, in_=cpa_d)
            cpb = consts.tile([32, 672], F16)
            nc.sync.dma_start(out=cpb, in_=cpb_d)
            st1 = cpa[:, 0:128]
            val1 = cpa[:, 128:192]
            msk1 = cpa[:, 192:208]
            val2 = cpb[:, 256:512]
            msk2 = cpb[:, 512:528]
            ob = cpb[0:1, 528:672]

            xt = xpool.tile([128, sizes[1], BL], F16, tag="xt")
            nc.sync.dma_start(out=xt, in_=xt_d[:, starts[1]:starts[1] + sizes[1], :])
            pre[1] = xt

            # ---- A1 table generation (g * A1[c, j, p%64] at [p, j, c]) ----
            # dg1[k, j, c] = val1[k, j] * msk1[k, c], then
            # a1[p, j, c] = sum_k st1[k, p] * dg1[k, j, c]
            dg1 = consts.tile([128, 64, C], F16)
            nc.vector.scalar_tensor_tensor(
                out=dg1,
                in0=val1.rearrange("p (m c) -> p m c", c=1)
                    .broadcast_to([128, 64, C]),
                scalar=1.0,
                in1=msk1.rearrange("p (m c) -> p m c", m=1)
                    .broadcast_to([128, 64, C]),
                op0=AOP.mult, op1=AOP.mult)
            a1 = consts.tile([128, 64, C], F16)
            for h in range(2):
                a1ps = gpspool.tile([128, 32, C], F32)
                nc.tensor.matmul(a1ps, st1, dg1[:, h * 32:(h + 1) * 32, :],
                                 start=True, stop=True)
                dst = a1[:, h * 32:(h + 1) * 32, :]
                if h == 0:
                    nc.scalar.copy(dst, a1ps)
                else:
                    nc.vector.tensor_scalar_add(dst, a1ps, 0.0)

            # ---- W2 dense generation: w2sb[p, nh, m, c] ----
            # dg[k, m, c] = val2[k, m] * msk2[k, c] (expanded per m-block)
            dg = consts.tile([32, D, C], F16)
            w2sb = consts.tile([128, 2, D, C], F16)
            for mblk in range(D // MBLK):
                ms = slice(mblk * MBLK, (mblk + 1) * MBLK)
                nc.vector.scalar_tensor_tensor(
                    out=dg[:, ms, :],
                    in0=val2[:, ms].rearrange("p (m c) -> p m c", c=1)
                        .broadcast_to([32, MBLK, C]),
                    scalar=1.0,
                    in1=msk2.rearrange("p (m c) -> p m c", m=1)
                        .broadcast_to([32, MBLK, C]),
                    op0=AOP.mult, op1=AOP.mult)
                for nh in range(2):
                    gps = gpspool.tile([128, MBLK, C], F32)
                    nc.tensor.matmul(gps, cpb[:, nh * 128:(nh + 1) * 128],
                                     dg[:, ms, :], start=True, stop=True)
                    dst = w2sb[:, nh, ms, :]
                    if nh == 0:
                        nc.scalar.copy(dst, gps)
                    else:
                        nc.vector.tensor_scalar_add(dst, gps, 0.0)

            # ---- main pass: scores accumulate over all 512 chunks ----
            sc = scpool.tile([BL, C], F32)
            first = True
            for gi in range(len(sizes)):
                if gi in pre:
                    xt = pre[gi]
                else:
                    xt = xpool.tile([128, sizes[gi], BL], F16, tag="xt")
                    nc.sync.dma_start(
                        out=xt,
                        in_=xt_d[:, starts[gi]:starts[gi] + sizes[gi], :])
                for t in range(sizes[gi]):
                    q = starts[gi] + t
                    m, nh = q // 2, q % 2
                    nc.tensor.matmul(sc, xt[:, t, :], w2sb[:, nh, m, :],
                                     start=first, stop=False)
                    first = False
                    nc.tensor.matmul(sc, xt[:, t, :], a1[:, m % 64, :],
                                     start=False, stop=False)
            nc.tensor.matmul(sc, ob[:, 0:BL], ob[:, BL:BL + C],
                             start=False, stop=True)

            # ---- softmax over the 16 free elements ----
            negmax = fin.tile([BL, 1], F32)
            nc.vector.tensor_reduce(out=negmax, in_=sc, axis=AXL.X,
                                    op=AOP.max, negate=True)
            e = fin.tile([BL, C], F32)
            sume = fin.tile([BL, 1], F32)
            nc.scalar.activation(out=e, in_=sc, func=AFT.Exp, bias=negmax,
                                 scale=1.0, accum_out=sume)
            rec = fin.tile([BL, 1], F32)
            nc.vector.reciprocal(rec, sume)
            probs = fin.tile([BL, C], F32)
            nc.vector.tensor_scalar_mul(probs, e, rec)
            nc.sync.dma_start(out=out_d, in_=probs)

    nc.compile()
    _cache["nc"] = nc
    return nc


def _host_prep(inputs, w1, w2, l, b):
    inputs = np.asarray(inputs, dtype=np.float32)
    w1 = np.asarray(w1, dtype=np.float32)
    w2 = np.asarray(w2, dtype=np.float32)
    l = np.asarray(l, dtype=np.float32)
    b = np.asarray(b, dtype=np.float32)

    g = np.float32(1.0 / (1.0 + np.exp(-np.float32(l[0]))))

    # A1 gen consts: st1[k=(c,r), p] = w1[c, p%64, r];
    # val1[k, j] = sign_r * g * w1[c, j, r]; msk1[k, c'] = (c' == k//8)
    w1t = w1.transpose(0, 2, 1)                                # [c, r, j]
    w1r = w1t.reshape(128, 64)
    st1 = np.concatenate([w1r, w1r], axis=1).astype(np.float16)
    signs = np.array([-1.0] * 4 + [1.0] * 4, np.float32)
    val1 = (w1t * g * signs[None, :, None]).reshape(128, 64)
    val1 = val1.astype(np.float16)
    msk1 = (np.arange(128)[:, None] // 8 ==
            np.arange(C)[None, :]).astype(np.float16)

    # W2 gen consts: st2[k, n] = (u2|v2)[k, n];
    # val2[k, m] = +-(1-g) * (u2|v2)[k, m]; msk2[k, c'] = (c' == k%16)
    u2, v2 = w2[:, :, 1], w2[:, :, 0]                          # [16, 256]
    st2 = np.concatenate([u2, v2], axis=0).astype(np.float16)  # [32, 256]
    val2 = np.concatenate([(1.0 - g) * u2, -(1.0 - g) * v2], axis=0)
    val2 = val2.astype(np.float16)
    msk2 = (np.arange(32)[:, None] % 16 ==
            np.arange(C)[None, :]).astype(np.float16)

    cpa = np.zeros((128, 256), np.float16)
    cpa[:, 0:128] = st1
    cpa[:, 128:192] = val1
    cpa[:, 192:208] = msk1
    cpb = np.zeros((32, 672), np.float16)
    cpb[:, 0:256] = st2
    cpb[:, 256:512] = val2
    cpb[:, 512:528] = msk2
    cpb[0, 528:656] = 1.0
    cpb[0, 656:672] = b.astype(np.float16)

    # x: [1024, 256, 256] -> per-core xt[p, q, s] = x[s, q*128 + p]
    xt_all = inputs.astype(np.float16).reshape(N_CORES, BL, NQ, 128)
    xt_all = np.ascontiguousarray(xt_all.transpose(0, 3, 2, 1))

    shared = dict(cpa=cpa, cpb=cpb)
    in_maps = []
    for core in range(N_CORES):
        m = dict(shared)
        m["xt"] = xt_all[core]
        in_maps.append(m)
    return in_maps


def kernel(inputs, w1, w2, l, b, _trace=False):
    nc = _build()
    in_maps = _host_prep(inputs, w1, w2, l, b)
    res = run_bass_kernel_spmd(nc, in_maps, core_ids=list(range(N_CORES)),
                               trace=_trace)
    out = np.concatenate([r["probs"] for r in res.results], axis=0)
    if _trace:
        kernel.last_results = res
    return out


# revision 21
# speedup vs baseline: 1.0509x; 1.0036x over previous
"""Trainium2 Bass kernel for nn_CLRBP_23124103922240.

Math: scores[s, c] = x[s] . W[c] + b[c], softmax over 16 classes, where
W[c] = g * tile4x4(A1[c]) + (1-g) * A2[c],
A1[c] = u1 u1^T - v1 v1^T (64x64, rank 8), A2[c] = u2 u2^T - v2 v2^T
(256x256, rank 2), g = sigmoid(l[0]).

Strategy (dense-W, X-stationary):
  - x is cast to fp16 on host (measured output rel-err 4.3e-3, gate 2e-2)
    and re-laid out per core as xt[p, q, s] = x[s, q*128 + p]: the flat
    65536-pixel axis is split into 512 chunks of 128; the DMA stream is
    16.8 MB/core, half of f32.
  - For each chunk q the X block [128 pixels, 128 samples] is the
    *stationary* matmul operand; two [128, 16] moving operands (the g*A1
    table slice and the (1-g)*W2 slice) accumulate scores [128 samples,
    16 classes] directly in PSUM.  No per-sample vector work at all.
  - W1-tiled never materializes: tile4x4 means the moving slice for chunk
    (m, nh) is g*A1[c, m%64, p%64], read from a host-precomputed
    [128, 64, 16] table (p-duplicated so no partition wrap is needed).
  - W2 (dense [65536, 16]) is generated on device from the rank-2 factors:
    for each row m and column half nh, out[p, c] = sum_k st2[k, nh*128+p]
    * dg[k, m, c] with st2 = [u2 | v2] and dg the per-m diag-packed
    (1-g)-scaled factors; 512 tiny matmuls + 16 PSUM->SBUF copies.
  - Bias enters as a K=1 matmul (ones x b); softmax on [128, 16] f32.

Data-parallel over 8 NeuronCores: batch 1024 -> 128 samples per core.
"""

import numpy as np

import concourse.bacc as bacc
import concourse.mybir as mybir
import concourse.tile as tile
from concourse.bass_utils import run_bass_kernel_spmd

N_CORES = 8
B, D, C = 1024, 256, 16
BL = B // N_CORES        # 128 samples per core
NQ = (D * D) // 128      # 512 pixel chunks
G = 16                   # chunks per x DMA group
NG = NQ // G             # 32 groups
MBLK = 32                # W2-gen rows per PSUM bank round

F16 = mybir.dt.float16
F32 = mybir.dt.float32
AOP = mybir.AluOpType
AFT = mybir.ActivationFunctionType
AXL = mybir.AxisListType

_cache = {}


def _build():
    if "nc" in _cache:
        return _cache["nc"]

    nc = bacc.Bacc("TRN2", target_bir_lowering=False, debug=False,
                   num_devices=N_CORES)

    xt_d = nc.dram_tensor("xt", [128, NQ, BL], F16, kind="ExternalInput").ap()
    # packed consts: cpa [:, 0:128]=st1, [:,128:192]=val1, [:,192:208]=msk1
    # cpb [0:32, 0:256]=st2, [:,256:512]=val2, [:,512:528]=msk2,
    #     [0:1, 528:672]=ones|bias
    cpa_d = nc.dram_tensor("cpa", [128, 256], F16, kind="ExternalInput").ap()
    cpb_d = nc.dram_tensor("cpb", [32, 672], F16, kind="ExternalInput").ap()
    out_d = nc.dram_tensor("probs", [BL, C], F32, kind="ExternalOutput").ap()

    with tile.TileContext(nc) as tc:
        with (
            tc.tile_pool(name="consts", bufs=1) as consts,
            tc.tile_pool(name="xp", bufs=8) as xpool,
            tc.tile_pool(name="fin", bufs=1) as fin,
            tc.tile_pool(name="gps", bufs=2, space="PSUM") as gpspool,
            tc.tile_pool(name="sc", bufs=1, space="PSUM") as scpool,
        ):
            # group schedule: big groups, tapered tail so the end-of-stream
            # drain only covers a couple of chunks
            sizes = [G] * (NQ // G - 1) + [8, 4, 2, 2]
            starts = [sum(sizes[:i]) for i in range(len(sizes))]

            # x DMA stream first so the big transfer starts at t=0; the
            # W2-gen consts (dg, st2) go right behind group 0 so generation
            # starts early
            pre = {}
            xt = xpool.tile([128, sizes[0], BL], F16, tag="xt")
            nc.sync.dma_start(out=xt, in_=xt_d[:, 0:sizes[0], :])
            pre[0] = xt

            cpa = consts.tile([128, 256], F16)
            nc.sync.dma_start(out=cpa, in_=cpa_d)

            xt = xpool.tile([128, sizes[1], BL], F16, tag="xt")
            nc.sync.dma_start(out=xt,
                              in_=xt_d[:, starts[1]:starts[1] + sizes[1], :])
            pre[1] = xt

            cpb = consts.tile([32, 672], F16)
            nc.sync.dma_start(out=cpb, in_=cpb_d)
            st1 = cpa[:, 0:128]
            val1 = cpa[:, 128:192]
            msk1 = cpa[:, 192:208]
            val2 = cpb[:, 256:512]
            msk2 = cpb[:, 512:528]
            ob = cpb[0:1, 528:672]

            # ---- A1 table generation (g * A1[c, j, p%64] at [p, j, c]) ----
            # dg1[k, j, c] = val1[k, j] * msk1[k, c], then
            # a1[p, j, c] = sum_k st1[k, p] * dg1[k, j, c]
            dg1 = consts.tile([128, 64, C], F16)
            nc.vector.scalar_tensor_tensor(
                out=dg1,
                in0=val1.rearrange("p (m c) -> p m c", c=1)
                    .broadcast_to([128, 64, C]),
                scalar=1.0,
                in1=msk1.rearrange("p (m c) -> p m c", m=1)
                    .broadcast_to([128, 64, C]),
                op0=AOP.mult, op1=AOP.mult)
            a1 = consts.tile([128, 64, C], F16)
            for h in range(2):
                a1ps = gpspool.tile([128, 32, C], F32)
                nc.tensor.matmul(a1ps, st1, dg1[:, h * 32:(h + 1) * 32, :],
                                 start=True, stop=True)
                dst = a1[:, h * 32:(h + 1) * 32, :]
                if h == 0:
                    nc.scalar.copy(dst, a1ps)
                else:
                    nc.vector.tensor_scalar_add(dst, a1ps, 0.0)

            # ---- W2 dense generation: w2sb[p, nh, m, c] ----
            # dg[k, m, c] = val2[k, m] * msk2[k, c] (expanded per m-block)
            dg = consts.tile([32, D, C], F16)
            w2sb = consts.tile([128, 2, D, C], F16)
            for mblk in range(D // MBLK):
                ms = slice(mblk * MBLK, (mblk + 1) * MBLK)
                nc.vector.scalar_tensor_tensor(
                    out=dg[:, ms, :],
                    in0=val2[:, ms].rearrange("p (m c) -> p m c", c=1)
                        .broadcast_to([32, MBLK, C]),
                    scalar=1.0,
                    in1=msk2.rearrange("p (m c) -> p m c", m=1)
                        .broadcast_to([32, MBLK, C]),
                    op0=AOP.mult, op1=AOP.mult)
                for nh in range(2):
                    gps = gpspool.tile([128, MBLK, C], F32)
                    nc.tensor.matmul(gps, cpb[:, nh * 128:(nh + 1) * 128],
                                     dg[:, ms, :], start=True, stop=True)
                    dst = w2sb[:, nh, ms, :]
                    if nh == 0:
                        nc.scalar.copy(dst, gps)
                    else:
                        nc.vector.tensor_scalar_add(dst, gps, 0.0)

            # ---- main pass: scores accumulate over all 512 chunks ----
            sc = scpool.tile([BL, C], F32)
            first = True
            for gi in range(len(sizes)):
                if gi in pre:
                    xt = pre[gi]
                else:
                    xt = xpool.tile([128, sizes[gi], BL], F16, tag="xt")
                    nc.sync.dma_start(
                        out=xt,
                        in_=xt_d[:, starts[gi]:starts[gi] + sizes[gi], :])
                for t in range(sizes[gi]):
                    q = starts[gi] + t
                    m, nh = q // 2, q % 2
                    nc.tensor.matmul(sc, xt[:, t, :], w2sb[:, nh, m, :],
                                     start=first, stop=False)
                    first = False
                    nc.tensor.matmul(sc, xt[:, t, :], a1[:, m % 64, :],
                                     start=False, stop=False)
            nc.tensor.matmul(sc, ob[:, 0:BL], ob[:, BL:BL + C],
                             start=False, stop=True)

            # ---- softmax over the 16 free elements ----
            negmax = fin.tile([BL, 1], F32)
            nc.vector.tensor_reduce(out=negmax, in_=sc, axis=AXL.X,
                                    op=AOP.max, negate=True)
            e = fin.tile([BL, C], F32)
            sume = fin.tile([BL, 1], F32)
            nc.scalar.activation(out=e, in_=sc, func=AFT.Exp, bias=negmax,
                                 scale=1.0, accum_out=sume)
            rec = fin.tile([BL, 1], F32)
            nc.vector.reciprocal(rec, sume)
            probs = fin.tile([BL, C], F32)
            nc.vector.tensor_scalar_mul(probs, e, rec)
            nc.sync.dma_start(out=out_d, in_=probs)

    nc.compile()
    _cache["nc"] = nc
    return nc


def _host_prep(inputs, w1, w2, l, b):
    inputs = np.asarray(inputs, dtype=np.float32)
    w1 = np.asarray(w1, dtype=np.float32)
    w2 = np.asarray(w2, dtype=np.float32)
    l = np.asarray(l, dtype=np.float32)
    b = np.asarray(b, dtype=np.float32)

    g = np.float32(1.0 / (1.0 + np.exp(-np.float32(l[0]))))

    # A1 gen consts: st1[k=(c,r), p] = w1[c, p%64, r];
    # val1[k, j] = sign_r * g * w1[c, j, r]; msk1[k, c'] = (c' == k//8)
    w1t = w1.transpose(0, 2, 1)                                # [c, r, j]
    w1r = w1t.reshape(128, 64)
    st1 = np.concatenate([w1r, w1r], axis=1).astype(np.float16)
    signs = np.array([-1.0] * 4 + [1.0] * 4, np.float32)
    val1 = (w1t * g * signs[None, :, None]).reshape(128, 64)
    val1 = val1.astype(np.float16)
    msk1 = (np.arange(128)[:, None] // 8 ==
            np.arange(C)[None, :]).astype(np.float16)

    # W2 gen consts: st2[k, n] = (u2|v2)[k, n];
    # val2[k, m] = +-(1-g) * (u2|v2)[k, m]; msk2[k, c'] = (c' == k%16)
    u2, v2 = w2[:, :, 1], w2[:, :, 0]                          # [16, 256]
    st2 = np.concatenate([u2, v2], axis=0).astype(np.float16)  # [32, 256]
    val2 = np.concatenate([(1.0 - g) * u2, -(1.0 - g) * v2], axis=0)
    val2 = val2.astype(np.float16)
    msk2 = (np.arange(32)[:, None] % 16 ==
            np.arange(C)[None, :]).astype(np.float16)

    cpa = np.zeros((128, 256), np.float16)
    cpa[:, 0:128] = st1
    cpa[:, 128:192] = val1
    cpa[:, 192:208] = msk1
    cpb = np.zeros((32, 672), np.float16)
    cpb[:, 0:256] = st2
    cpb[:, 256:512] = val2
    cpb[:, 512:528] = msk2
    cpb[0, 528:656] = 1.0
    cpb[0, 656:672] = b.astype(np.float16)

    # x: [1024, 256, 256] -> per-core xt[p, q, s] = x[s, q*128 + p]
    xt_all = inputs.astype(np.float16).reshape(N_CORES, BL, NQ, 128)
    xt_all = np.ascontiguousarray(xt_all.transpose(0, 3, 2, 1))

    shared = dict(cpa=cpa, cpb=cpb)
    in_maps = []
    for core in range(N_CORES):
        m = dict(shared)
        m["xt"] = xt_all[core]
        in_maps.append(m)
    return in_maps


def kernel(inputs, w1, w2, l, b, _trace=False):
    nc = _build()
    in_maps = _host_prep(inputs, w1, w2, l, b)
    res = run_bass_kernel_spmd(nc, in_maps, core_ids=list(range(N_CORES)),
                               trace=_trace)
    out = np.concatenate([r["probs"] for r in res.results], axis=0)
    if _trace:
        kernel.last_results = res
    return out
